# revision 9
# baseline (speedup 1.0000x reference)
"""Trainium2 Bass kernel for LoRALayer: out = 2.0 * (x @ B) @ A.

x: [4, 4096, 4096] f32; A: [8, 4096] f32; B: [4096, 8] f32.
Sharding: data-parallel on the 16384 tokens across 8 cores (2048 each);
A/B replicated. Host-side prep (part of sharding): each core's x-shard is
shipped transposed (contraction dim on SBUF partitions) as a single bf16
stream; B and 2*A likewise bf16. Output leaves the device as bf16 and the
host upconverts to f32 during the gather. Total HBM traffic is 32 MiB/core
(the kernel is HBM-bound); bf16 rounding of x dominates the ~7e-3
absmax-rel error (gate 2e-2).

Per core (T=2048), per 256-token block, PE work is packed with tile_position
concurrency so the PE never gates the DMA streams:
  mm1 (2x col-tiled): even feature chunks accumulate into ps_y[0:8],
      odd chunks into ps_y[32:40] (independent PE column groups, separate
      xbus streams) -> 16 rounds of 2 concurrent 128x8x256 matmuls.
  y   = even+odd strips, split into two 128-token halves at partition
      bases 0/32 (bf16), matching A2 replicated at bases 0/32.
  mm2 (2x row-tiled): per 512-col chunk of A2, two concurrent rank-8
      matmuls (row groups 0/1) -> two PSUM banks; ACT copies subtile 0,
      DVE copies subtile 1 (PSUM->SBUF bf16).
Input DMAs ride the sync HWDGE ring, output DMAs the gpsimd (SWDGE) ring
so trigger issue never serializes with copies.
"""

import numpy as np

P = 128
F_IN = 4096
F_OUT = 4096
RANK = 8
N_CORES = 8
SCALING = 2.0
TBLK = 256             # token block: 2 subtiles of 128 tokens

_CACHE = {}


def _build_nc(T, F_in, F_out, R):
    """Build the single-core Bass program for a T-token shard."""
    from contextlib import ExitStack

    import concourse.mybir as mybir
    import concourse.tile as tile
    from concourse import bacc

    f32 = mybir.dt.float32
    bf16 = mybir.dt.bfloat16
    tblk = min(TBLK, T)
    CH = F_in // P          # feature chunks (32)
    NB = T // tblk          # token blocks (8)
    NSUB = tblk // P        # 128-token subtiles per block (2)
    NS = F_out // 512       # output column chunks (8)
    CGRP = CH // NSUB       # chunks per input sub-DMA (16 -> 1MB granularity)
    RB = 32                 # partition-base alignment for engine APs

    nc = bacc.Bacc("TRN2", target_bir_lowering=False, debug=False)

    xh_d = nc.dram_tensor(
        "xh", [NB, NSUB, P, CGRP * tblk], bf16, kind="ExternalInput"
    ).ap()
    bpk_d = nc.dram_tensor("Bpk", [P, CH * R], bf16, kind="ExternalInput").ap()
    a2_d = nc.dram_tensor("A2", [R, F_out], bf16, kind="ExternalInput").ap()
    out_d = nc.dram_tensor("out", [T, F_out], bf16, kind="ExternalOutput").ap()

    with tile.TileContext(nc) as tc, ExitStack() as ctx:
        cpool = ctx.enter_context(tc.tile_pool(name="const", bufs=1))
        xtpool = ctx.enter_context(tc.tile_pool(name="xt", bufs=3 * NSUB))
        ypool = ctx.enter_context(tc.tile_pool(name="yt", bufs=2))
        opool = ctx.enter_context(tc.tile_pool(name="osb", bufs=6))
        y_pp = ctx.enter_context(tc.tile_pool(name="y_ps", bufs=2, space="PSUM"))
        o_pp = ctx.enter_context(tc.tile_pool(name="o_ps", bufs=2, space="PSUM"))

        bpk_sb = cpool.tile([P, CH * R], bf16, tag="bpk_sb")
        nc.sync.dma_start(bpk_sb[:], bpk_d)
        # A2 replicated at partition bases 0 and 32 for the row-tiled mm2.
        a2_sb = cpool.tile([RB + R, F_out], bf16, tag="a2_sb")
        nc.sync.dma_start(a2_sb[:R, :], a2_d)
        nc.sync.dma_start(a2_sb[RB:RB + R, :], a2_d)

        state = {}

        for blk in range(NB + 1):
            xts = []
            if blk < NB:
                # Input sub-DMAs alternate the sync and scalar HWDGE queues:
                # each queue drives 8 SDMA engines (~216 GB/s), so two queues
                # are needed to reach the ~435 GB/s fabric ceiling.
                for s in range(NSUB):
                    x_sb = xtpool.tile([P, CGRP, tblk], bf16, tag="x_sb")
                    eng = nc.sync if s % 2 == 0 else nc.scalar
                    eng.dma_start(
                        x_sb[:].rearrange("p c t -> p (c t)"), xh_d[blk, s]
                    )
                    xts.append(x_sb)
                ps_y = y_pp.tile([RB + R, 512], f32, tag="ps_y")

            for h in range(NSUB):
                if blk > 0:
                    y_pk, o_sbs = state[blk - 1]
                    if h == 0:
                        o_sbs.append(opool.tile(
                            [P, F_out], bf16, tag="o_sb0",
                            name=f"o_sb0_{blk}",
                        ))
                        o_sbs.append(opool.tile(
                            [P, F_out], bf16, tag="o_sb1",
                            name=f"o_sb1_{blk}",
                        ))
                    for n in range(h * NS // NSUB, (h + 1) * NS // NSUB):
                        cs = slice(n * 512, (n + 1) * 512)
                        o_ps0 = o_pp.tile([P, 512], f32, tag="o_ps0")
                        o_ps1 = o_pp.tile([P, 512], f32, tag="o_ps1")
                        nc.tensor.matmul(
                            o_ps0[:], y_pk[:R, :], a2_sb[:R, cs],
                            start=True, stop=True,
                        )
                        nc.tensor.matmul(
                            o_ps1[:], y_pk[RB:RB + R, :], a2_sb[RB:RB + R, cs],
                            start=True, stop=True,
                        )
                        nc.scalar.copy(o_sbs[0][:, cs], o_ps0[:])
                        nc.vector.tensor_copy(o_sbs[1][:, cs], o_ps1[:])
                if blk < NB:
                    for r in range(h * CGRP // 2, (h + 1) * CGRP // 2):
                        c0, c1 = 2 * r, 2 * r + 1
                        nc.tensor.matmul(
                            ps_y[:R, :tblk],
                            bpk_sb[:, c0 * R:(c0 + 1) * R],
                            xts[c0 // CGRP][:, c0 % CGRP, :],
                            start=(r == 0), stop=(r == CH // 2 - 1),
                        )
                        nc.tensor.matmul(
                            ps_y[RB:RB + R, :tblk],
                            bpk_sb[:, c1 * R:(c1 + 1) * R],
                            xts[c1 // CGRP][:, c1 % CGRP, :],
                            start=(r == 0), stop=(r == CH // 2 - 1),
                        )
            if blk > 0:
                # o_sb0 rides the scalar queue right behind its own ACT
                # copies (wait already satisfied); o_sb1 rides the gpsimd
                # queue so the output stream spans two queues on the drain.
                _, o_sbs = state.pop(blk - 1)
                trow = (blk - 1) * tblk
                nc.scalar.dma_start(out_d[trow:trow + P, :], o_sbs[0][:])
                nc.gpsimd.dma_start(out_d[trow + P:trow + 2 * P, :], o_sbs[1][:])
            if blk < NB:
                # y = even+odd col-tile strips; two 128-token halves at
                # partition bases 0/32 (the mm2 row-tile weight layout).
                # DVE reads at most one PSUM operand -> stage strip 0 in SBUF.
                yt = ypool.tile([R, tblk], f32, tag="yt")
                nc.vector.tensor_copy(yt[:], ps_y[:R, :tblk])
                y_pk = ypool.tile([RB + R, P], bf16, tag="y_pk")
                nc.vector.tensor_add(
                    y_pk[:R, :], yt[:, :P], ps_y[RB:RB + R, :P]
                )
                nc.vector.tensor_add(
                    y_pk[RB:RB + R, :], yt[:, P:tblk], ps_y[RB:RB + R, P:tblk]
                )
                state[blk] = (y_pk, [])

    nc.compile()
    return nc


def _pack_inputs(x2d, A, B, T_shard, F_in, R):
    """Shard x on tokens (transposed, bf16); replicate B/A2 packs."""
    import ml_dtypes

    bf16 = ml_dtypes.bfloat16
    CH = F_in // P

    Bb = B.astype(np.float32).astype(bf16)
    bpk = np.ascontiguousarray(
        Bb.reshape(CH, P, R).transpose(1, 0, 2).reshape(P, CH * R)
    )
    a2 = np.ascontiguousarray((SCALING * A.astype(np.float32)).astype(bf16))

    T = T_shard
    tblk = min(TBLK, T)
    NB = T // tblk
    NSUB = tblk // P
    CGRP = CH // NSUB

    def pack(m):
        a = m.reshape(NSUB, CGRP, P, NB, tblk)
        a = a.transpose(3, 0, 2, 1, 4)
        return np.ascontiguousarray(a.reshape(NB, NSUB, P, CGRP * tblk))

    n_shards = x2d.shape[0] // T_shard
    in_maps = []
    for c in range(n_shards):
        xt = np.ascontiguousarray(x2d[c * T_shard:(c + 1) * T_shard].T)
        in_maps.append({"xh": pack(xt.astype(bf16)), "Bpk": bpk, "A2": a2})
    return in_maps


def kernel(x, A, B):
    from concourse.bass_utils import run_bass_kernel_spmd

    x = np.asarray(x, dtype=np.float32)
    A = np.asarray(A, dtype=np.float32)
    B = np.asarray(B, dtype=np.float32)
    orig_shape = x.shape
    x2d = x.reshape(-1, F_IN)
    T_shard = x2d.shape[0] // N_CORES

    key = (T_shard, F_IN, F_OUT, RANK)
    if key not in _CACHE:
        _CACHE[key] = _build_nc(T_shard, F_IN, F_OUT, RANK)
    nc = _CACHE[key]

    in_maps = _pack_inputs(x2d, A, B, T_shard, F_IN, RANK)
    res = run_bass_kernel_spmd(nc, in_maps, core_ids=list(range(N_CORES)))
    out = np.concatenate(
        [np.asarray(r["out"], dtype=np.float32) for r in res.results], axis=0
    )
    return out.reshape(*orig_shape[:-1], F_OUT)


# revision 10
# speedup vs baseline: 1.1879x; 1.1879x over previous
"""Trainium2 Bass kernel for LoRALayer: out = 2.0 * (x @ B) @ A.

x: [4, 4096, 4096] f32; A: [8, 4096] f32; B: [4096, 8] f32.
Sharding: data-parallel on the 16384 tokens across 8 cores (2048 each);
A/B replicated. Host-side prep (part of sharding): each core's x-shard is
shipped transposed (contraction dim on SBUF partitions) as a single bf16
stream; B and 2*A likewise bf16. Output leaves the device as bf16 and the
host upconverts to f32 during the gather. Total HBM traffic is 32 MiB/core
and the kernel is fabric-bound (~435 GB/s/core SBUF AXI, ~216 GB/s per
DGE queue); bf16 rounding of x dominates the ~7e-3 absmax-rel error
(gate 2e-2).

Per core (T=2048), per 256-token block, PE work is packed with
tile_position concurrency so the (HAM-cold) PE never gates the streams:
  mm1 (4x col-tiled): feature chunk 4r+j accumulates into ps_y[32j:32j+8]
      (independent PE column groups, separate xbus streams) -> 8 rounds of
      4 concurrent 128x8x256 matmuls.
  y   = sum of 4 strips, split into two 128-token halves at partition
      bases 0/32 (bf16), matching A2 replicated at bases 0/32.
  mm2 (2x row-tiled): per 512-col chunk of A2, two concurrent rank-8
      matmuls (row groups 0/1) -> two PSUM banks; ACT copies subtile 0,
      DVE copies subtile 1 (PSUM->SBUF bf16).
DMA queue balance (each DGE queue drives 8 SDMA engines, ~216 GB/s):
  sync   = in0 + in1-on-odd-blocks   (12.6 MB, input only, no coupling)
  gpsimd = in1-on-even-blocks + out1 (12.6 MB)
  scalar = out0 only, emitted right after its own ACT copies (8.4 MB,
           waits trivially satisfied -> no head-of-line stalls)
"""

import numpy as np

P = 128
F_IN = 4096
F_OUT = 4096
RANK = 8
N_CORES = 8
SCALING = 2.0
TBLK = 256             # token block: 2 subtiles of 128 tokens

_CACHE = {}


def _build_nc(T, F_in, F_out, R):
    """Build the single-core Bass program for a T-token shard."""
    from contextlib import ExitStack

    import concourse.mybir as mybir
    import concourse.tile as tile
    from concourse import bacc

    f32 = mybir.dt.float32
    bf16 = mybir.dt.bfloat16
    tblk = min(TBLK, T)
    CH = F_in // P          # feature chunks (32)
    NB = T // tblk          # token blocks (8)
    NSUB = tblk // P        # 128-token subtiles per block (2)
    NS = F_out // 512       # output column chunks (8)
    CGRP = CH // NSUB       # chunks per input sub-DMA (16 -> 1MB granularity)
    RB = 32                 # partition-base alignment for engine APs
    CT = 4                  # mm1 column-tiling width

    nc = bacc.Bacc("TRN2", target_bir_lowering=False, debug=False)

    xh_d = nc.dram_tensor(
        "xh", [NB, NSUB, P, CGRP * tblk], bf16, kind="ExternalInput"
    ).ap()
    bpk_d = nc.dram_tensor("Bpk", [P, CH * R], bf16, kind="ExternalInput").ap()
    a2_d = nc.dram_tensor("A2", [R, F_out], bf16, kind="ExternalInput").ap()
    out_d = nc.dram_tensor("out", [T, F_out], bf16, kind="ExternalOutput").ap()

    with tile.TileContext(nc) as tc, ExitStack() as ctx:
        cpool = ctx.enter_context(tc.tile_pool(name="const", bufs=1))
        xtpool = ctx.enter_context(tc.tile_pool(name="xt", bufs=3 * NSUB))
        ypool = ctx.enter_context(tc.tile_pool(name="yt", bufs=2))
        opool = ctx.enter_context(tc.tile_pool(name="osb", bufs=6))
        y_pp = ctx.enter_context(tc.tile_pool(name="y_ps", bufs=2, space="PSUM"))
        o_pp = ctx.enter_context(tc.tile_pool(name="o_ps", bufs=2, space="PSUM"))

        bpk_sb = cpool.tile([P, CH * R], bf16, tag="bpk_sb")
        nc.sync.dma_start(bpk_sb[:], bpk_d)
        # A2 replicated at partition bases 0 and 32 for the row-tiled mm2.
        a2_sb = cpool.tile([RB + R, F_out], bf16, tag="a2_sb")
        nc.sync.dma_start(a2_sb[:R, :], a2_d)
        nc.sync.dma_start(a2_sb[RB:RB + R, :], a2_d)

        state = {}

        for blk in range(NB + 1):
            xts = []
            if blk < NB:
                for s in range(NSUB):
                    x_sb = xtpool.tile([P, CGRP, tblk], bf16, tag="x_sb")
                    eng = nc.sync if (s == 0 or blk % 2 == 1) else nc.gpsimd
                    eng.dma_start(
                        x_sb[:].rearrange("p c t -> p (c t)"), xh_d[blk, s]
                    )
                    xts.append(x_sb)
                ps_y = y_pp.tile([3 * RB + R, 512], f32, tag="ps_y")

            for h in range(NSUB):
                if blk > 0:
                    y_pk, o_sbs = state[blk - 1]
                    if h == 0:
                        o_sbs.append(opool.tile(
                            [P, F_out], bf16, tag="o_sb0",
                            name=f"o_sb0_{blk}",
                        ))
                        o_sbs.append(opool.tile(
                            [P, F_out], bf16, tag="o_sb1",
                            name=f"o_sb1_{blk}",
                        ))
                    for n in range(h * NS // NSUB, (h + 1) * NS // NSUB):
                        cs = slice(n * 512, (n + 1) * 512)
                        o_ps0 = o_pp.tile([P, 512], f32, tag="o_ps0")
                        o_ps1 = o_pp.tile([P, 512], f32, tag="o_ps1")
                        nc.tensor.matmul(
                            o_ps0[:], y_pk[:R, :], a2_sb[:R, cs],
                            start=True, stop=True,
                        )
                        nc.tensor.matmul(
                            o_ps1[:], y_pk[RB:RB + R, :], a2_sb[RB:RB + R, cs],
                            start=True, stop=True,
                        )
                        nc.scalar.copy(o_sbs[0][:, cs], o_ps0[:])
                        nc.vector.tensor_copy(o_sbs[1][:, cs], o_ps1[:])
                    if h == NSUB - 1:
                        trow = (blk - 1) * tblk
                        nc.scalar.dma_start(
                            out_d[trow:trow + P, :], o_sbs[0][:]
                        )
                        nc.gpsimd.dma_start(
                            out_d[trow + P:trow + 2 * P, :], o_sbs[1][:]
                        )
                        del state[blk - 1]
                if blk < NB:
                    for r in range(h * CH // CT // NSUB, (h + 1) * CH // CT // NSUB):
                        for j in range(CT):
                            c = CT * r + j
                            nc.tensor.matmul(
                                ps_y[j * RB:j * RB + R, :tblk],
                                bpk_sb[:, c * R:(c + 1) * R],
                                xts[c // CGRP][:, c % CGRP, :],
                                start=(r == 0), stop=(r == CH // CT - 1),
                                tile_position=(0, j * RB),
                            )
            if blk < NB:
                # y = sum of the 4 col-tile strips; two 128-token halves at
                # partition bases 0/32 (the mm2 row-tile weight layout).
                # DVE reads at most one PSUM operand per op -> stage in SBUF.
                yt = ypool.tile([R, tblk], f32, tag="yt")
                nc.vector.tensor_copy(yt[:], ps_y[:R, :tblk])
                for j in range(1, CT - 1):
                    nc.vector.tensor_add(
                        yt[:], yt[:], ps_y[j * RB:j * RB + R, :tblk]
                    )
                j = CT - 1
                y_pk = ypool.tile([RB + R, P], bf16, tag="y_pk")
                nc.vector.tensor_add(
                    y_pk[:R, :], yt[:, :P], ps_y[j * RB:j * RB + R, :P]
                )
                nc.vector.tensor_add(
                    y_pk[RB:RB + R, :], yt[:, P:tblk],
                    ps_y[j * RB:j * RB + R, P:tblk]
                )
                state[blk] = (y_pk, [])

    nc.compile()
    return nc


def _pack_inputs(x2d, A, B, T_shard, F_in, R):
    """Shard x on tokens (transposed, bf16); replicate B/A2 packs."""
    import ml_dtypes

    bf16 = ml_dtypes.bfloat16
    CH = F_in // P

    Bb = B.astype(np.float32).astype(bf16)
    bpk = np.ascontiguousarray(
        Bb.reshape(CH, P, R).transpose(1, 0, 2).reshape(P, CH * R)
    )
    a2 = np.ascontiguousarray((SCALING * A.astype(np.float32)).astype(bf16))

    T = T_shard
    tblk = min(TBLK, T)
    NB = T // tblk
    NSUB = tblk // P
    CGRP = CH // NSUB

    def pack(m):
        a = m.reshape(NSUB, CGRP, P, NB, tblk)
        a = a.transpose(3, 0, 2, 1, 4)
        return np.ascontiguousarray(a.reshape(NB, NSUB, P, CGRP * tblk))

    n_shards = x2d.shape[0] // T_shard
    in_maps = []
    for c in range(n_shards):
        xt = np.ascontiguousarray(x2d[c * T_shard:(c + 1) * T_shard].T)
        in_maps.append({"xh": pack(xt.astype(bf16)), "Bpk": bpk, "A2": a2})
    return in_maps


def kernel(x, A, B):
    from concourse.bass_utils import run_bass_kernel_spmd

    x = np.asarray(x, dtype=np.float32)
    A = np.asarray(A, dtype=np.float32)
    B = np.asarray(B, dtype=np.float32)
    orig_shape = x.shape
    x2d = x.reshape(-1, F_IN)
    T_shard = x2d.shape[0] // N_CORES

    key = (T_shard, F_IN, F_OUT, RANK)
    if key not in _CACHE:
        _CACHE[key] = _build_nc(T_shard, F_IN, F_OUT, RANK)
    nc = _CACHE[key]

    in_maps = _pack_inputs(x2d, A, B, T_shard, F_IN, RANK)
    res = run_bass_kernel_spmd(nc, in_maps, core_ids=list(range(N_CORES)))
    out = np.concatenate(
        [np.asarray(r["out"], dtype=np.float32) for r in res.results], axis=0
    )
    return out.reshape(*orig_shape[:-1], F_OUT)
